# revision 5
# baseline (speedup 1.0000x reference)
"""Trainium2 Bass kernel for nn_EnsembleModel histogram_binning.

Math (reference):
    pair_idx[b,p1,p2] = adds[b,p1]*1024 + adds[b,p2]
    hist = segment_sum(a_arc, pair_idx, 1024*1024)       # scatter-add
    score = sigmoid(hist)
    out[b,p1,p2] = s_arc[b,p1,p2] + 0.3*score[pos[b,p1]*1024 + pos[b,p2]]

Device formulation (per batch b, all matmuls on the PE at fp32r):
    onehotA[p,v]  = (adds[b,p]==v)             [1024,1024]
    Gt   = a_arc[b].T @ onehotA                (lhsT=a_arc -> out[p2,v1])
    hist2d += Gt.T @ onehotA                   (lhsT=Gt    -> out[v1,v2])
    -- AllReduce(hist2d) over 8 cores, score2d = sigmoid(hist2d) --
    onehotPT[v,p] = (pos[b,p]==v)              [1024,1024]
    Wt   = score2d.T(gathered) : lhsT=score2d, rhs=onehotPT -> out[v2,p1]
    T    = lhsT=Wt, rhs=onehotPT               -> out[p1,p2] = score2d[pos[p1],pos[p2]]
    out  = s_arc[b] + (ALPHA folded into Wt copy) * T

Data parallel over batch: core c owns batches [4c, 4c+4).
"""

import sys

import numpy as np

_TRN_REPO = "/opt/trn_rl_repo"
if _TRN_REPO not in sys.path:
    sys.path.insert(0, _TRN_REPO)

import concourse.bass as bass  # noqa: E402
import concourse.mybir as mybir  # noqa: E402
from concourse import bacc  # noqa: E402
from concourse.tile import TileContext  # noqa: E402
from concourse.bass_utils import run_bass_kernel_spmd  # noqa: E402

N_POS = 1024
ALPHA = 0.3
B, S = 32, 1024
NCORES = 8
BPC = B // NCORES  # batches per core
KB = S // 128  # 8 k-blocks of 128
F32 = mybir.dt.float32
F32R = mybir.dt.float32r
I32 = mybir.dt.int32

_CACHE = {}


def _build_nc():
    EQ = mybir.AluOpType.is_equal
    ADD = mybir.AluOpType.add
    nc = bacc.Bacc("TRN2", target_bir_lowering=False, debug=False, num_devices=NCORES)
    a_d = nc.dram_tensor("a", [BPC, S, S], F32R, kind="ExternalInput")
    s_d = nc.dram_tensor("s", [BPC, S, S], F32, kind="ExternalInput")
    adds_d = nc.dram_tensor("adds", [BPC, S], F32, kind="ExternalInput")
    pos_d = nc.dram_tensor("pos", [BPC, S], F32, kind="ExternalInput")
    o_d = nc.dram_tensor("o", [BPC, S, S], F32, kind="ExternalOutput")
    hist_in = nc.dram_tensor("hist_in", [N_POS, N_POS], F32)
    hist_out = nc.dram_tensor("hist_out", [N_POS, N_POS], F32, addr_space="Shared")

    with TileContext(nc) as tc:
        with (
            tc.tile_pool(name="const", bufs=1) as pconst,
            tc.tile_pool(name="psum", bufs=8, space="PSUM") as pps,
        ):
            iota_v = pconst.tile([128, S], F32)
            nc.gpsimd.iota(
                iota_v[:],
                pattern=[[1, S]],
                base=0,
                channel_multiplier=0,
                allow_small_or_imprecise_dtypes=True,
            )
            iota2 = pconst.tile([128, KB], F32)
            nc.gpsimd.iota(
                iota2[:],
                pattern=[[128, KB]],
                base=0,
                channel_multiplier=1,
                allow_small_or_imprecise_dtypes=True,
            )

            # ---------------- Phase A: local histogram ----------------
            with (
                tc.tile_pool(name="pa", bufs=2) as pa,
                tc.tile_pool(name="poh", bufs=2) as poh,
                tc.tile_pool(name="pgt", bufs=1) as pgt,
                tc.tile_pool(name="phist", bufs=1) as phist,
                tc.tile_pool(name="pidx", bufs=2) as pidx,
            ):
                hist_sb = phist.tile([128, KB, N_POS], F32)
                for b in range(BPC):
                    adds_col = pidx.tile([128, KB], F32, tag="addsc")
                    nc.sync.dma_start(
                        adds_col[:], adds_d.ap()[b].rearrange("(mb q) -> q mb", q=128)
                    )
                    a_t = pa.tile([128, KB, S], F32R, tag="a")
                    nc.sync.dma_start(
                        a_t[:], a_d.ap()[b].rearrange("(kb kp) n -> kp kb n", kp=128)
                    )
                    oh = poh.tile([128, KB, N_POS], F32R, tag="oh")
                    for pb in range(KB):
                        nc.vector.tensor_scalar(
                            oh[:, pb, :], iota_v[:], adds_col[:, pb : pb + 1], None, op0=EQ
                        )
                    # MM1: Gt[p2, v1] = sum_p1 a[p1,p2] * onehotA[p1,v1]
                    gt = pgt.tile([128, KB, S], F32R, tag="gt")
                    for m in range(KB):
                        for n in range(2):
                            ps = pps.tile([128, 512], F32, tag="ps")
                            for k in range(KB):
                                nc.tensor.matmul(
                                    ps[:],
                                    a_t[:, k, m * 128 : (m + 1) * 128],
                                    oh[:, k, n * 512 : (n + 1) * 512],
                                    start=(k == 0),
                                    stop=(k == KB - 1),
                                )
                            nc.scalar.copy(gt[:, m, n * 512 : (n + 1) * 512], ps[:])
                    # MM2: hist2d[v1, v2] += sum_p2 Gt[p2,v1] * onehotA[p2,v2]
                    for m in range(KB):
                        for n in range(2):
                            ph = pps.tile([128, 512], F32, tag="ps")
                            for k in range(KB):
                                nc.tensor.matmul(
                                    ph[:],
                                    gt[:, k, m * 128 : (m + 1) * 128],
                                    oh[:, k, n * 512 : (n + 1) * 512],
                                    start=(k == 0),
                                    stop=(k == KB - 1),
                                )
                            dst = hist_sb[:, m, n * 512 : (n + 1) * 512]
                            if b == 0:
                                nc.vector.tensor_copy(dst, ph[:])
                            else:
                                nc.vector.tensor_tensor(dst, dst, ph[:], op=ADD)
                nc.sync.dma_start(
                    hist_in.ap().rearrange("(mb q) v -> q mb v", q=128), hist_sb[:]
                )

            nc.gpsimd.collective_compute(
                "AllReduce",
                ADD,
                replica_groups=[list(range(NCORES))],
                ins=[hist_in[:]],
                outs=[hist_out[:]],
            )

            # ---------------- sigmoid ----------------
            with tc.tile_pool(name="pscore", bufs=1) as pscore:
                score_t = pscore.tile([128, KB, N_POS], F32R)
                with tc.tile_pool(name="praw", bufs=1) as praw:
                    raw = praw.tile([128, KB, N_POS], F32)
                    nc.sync.dma_start(
                        raw[:], hist_out.ap().rearrange("(kb q) v -> q kb v", q=128)
                    )
                    nc.scalar.activation(
                        score_t[:], raw[:], mybir.ActivationFunctionType.Sigmoid
                    )

                # ---------------- Phase B: gather + add ----------------
                with (
                    tc.tile_pool(name="pohT", bufs=2) as pohT,
                    tc.tile_pool(name="pwt", bufs=1) as pwt,
                    tc.tile_pool(name="pst", bufs=2) as pst,
                    tc.tile_pool(name="pposb", bufs=1) as pposb,
                    tc.tile_pool(name="pout", bufs=3) as pout,
                ):
                    for b in range(BPC):
                        pos_b = pposb.tile([128, S], F32, tag="posb")
                        nc.sync.dma_start(
                            pos_b[:],
                            pos_d.ap()[b].unsqueeze(0).partition_broadcast(128),
                        )
                        ohT = pohT.tile([128, KB, S], F32R, tag="ohT")
                        for c in range(KB):
                            nc.vector.tensor_scalar(
                                ohT[:, c, :], pos_b[:], iota2[:, c : c + 1], None, op0=EQ
                            )
                        # MMa: Wt[v2, p1] = sum_v1 score2d[v1,v2] * onehotPT[v1,p1]
                        wt = pwt.tile([128, KB, S], F32R, tag="wt")
                        for m in range(KB):
                            for n in range(2):
                                pw = pps.tile([128, 512], F32, tag="ps")
                                for k in range(KB):
                                    nc.tensor.matmul(
                                        pw[:],
                                        score_t[:, k, m * 128 : (m + 1) * 128],
                                        ohT[:, k, n * 512 : (n + 1) * 512],
                                        start=(k == 0),
                                        stop=(k == KB - 1),
                                    )
                                # fold ALPHA into the PSUM->SBUF copy
                                nc.scalar.mul(
                                    wt[:, m, n * 512 : (n + 1) * 512], pw[:], ALPHA
                                )
                        s_t = pst.tile([128, KB, S], F32, tag="st")
                        nc.sync.dma_start(
                            s_t[:], s_d.ap()[b].rearrange("(mb q) n -> q mb n", q=128)
                        )
                        # MMb: T[p1, p2] = sum_v2 Wt[v2,p1] * onehotPT[v2,p2]
                        o_view = o_d.ap()[b].rearrange("(mb q) n -> q mb n", q=128)
                        for m in range(KB):
                            for n in range(2):
                                pt = pps.tile([128, 512], F32, tag="ps")
                                for k in range(KB):
                                    nc.tensor.matmul(
                                        pt[:],
                                        wt[:, k, m * 128 : (m + 1) * 128],
                                        ohT[:, k, n * 512 : (n + 1) * 512],
                                        start=(k == 0),
                                        stop=(k == KB - 1),
                                    )
                                ot = pout.tile([128, 512], F32, tag="ot")
                                nc.vector.tensor_tensor(
                                    ot[:],
                                    s_t[:, m, n * 512 : (n + 1) * 512],
                                    pt[:],
                                    op=ADD,
                                )
                                nc.sync.dma_start(
                                    o_view[:, m, n * 512 : (n + 1) * 512], ot[:]
                                )
    nc.finalize()
    return nc


def _get_nc():
    if "nc" not in _CACHE:
        _CACHE["nc"] = _build_nc()
    return _CACHE["nc"]


def _make_in_maps(s_arc, a_arc, adds, pos):
    in_maps = []
    for c in range(NCORES):
        sl = slice(BPC * c, BPC * (c + 1))
        in_maps.append(
            {
                "a": np.ascontiguousarray(a_arc[sl], dtype=np.float32),
                "s": np.ascontiguousarray(s_arc[sl], dtype=np.float32),
                "adds": np.ascontiguousarray(adds[sl], dtype=np.float32),
                "pos": np.ascontiguousarray(pos[sl], dtype=np.float32),
            }
        )
    return in_maps


def _run(in_maps, trace=False, **kwargs):
    return run_bass_kernel_spmd(
        _get_nc(), in_maps, core_ids=list(range(NCORES)), trace=trace, **kwargs
    )


def kernel(s_arc, a_arc, adds, pos):
    s_arc = np.asarray(s_arc)
    a_arc = np.asarray(a_arc)
    assert s_arc.shape == (B, S, S) and a_arc.shape == (B, S, S), (
        s_arc.shape,
        a_arc.shape,
    )
    res = _run(_make_in_maps(s_arc, a_arc, adds, pos), trace=False)
    out = np.empty((B, S, S), dtype=np.float32)
    for c in range(NCORES):
        out[BPC * c : BPC * (c + 1)] = res.results[c]["o"]
    return out


# revision 6
# speedup vs baseline: 1.1167x; 1.1167x over previous
"""Trainium2 Bass kernel for nn_EnsembleModel histogram_binning.

Math (reference):
    pair_idx[b,p1,p2] = adds[b,p1]*1024 + adds[b,p2]
    hist = segment_sum(a_arc, pair_idx, 1024*1024)       # scatter-add
    score = sigmoid(hist)
    out[b,p1,p2] = s_arc[b,p1,p2] + 0.3*score[pos[b,p1]*1024 + pos[b,p2]]

Device formulation, data-parallel over batch (core c owns batches [4c,4c+4)):
  Phase A (per local batch, fp32r matmuls on the PE):
    onehotA[p,v] = (adds[b,p]==v)              [1024,1024]
    Gt   = a_arc[b].T @ onehotA                (lhsT=a_arc -> out[p2,v1])
    hist2d += Gt.T @ onehotA                   (lhsT=Gt    -> out[v1,v2])
  AllReduce(hist2d) over 8 cores, split in two 512-row halves so the first
  half reduces while the second is still being computed.
  score_bf = bf16(ALPHA * sigmoid(hist2d)) staged to DRAM.
  Phase B (per local batch):
    Wt[v2,p1] = score_bf[pos[p1], v2]   via 2x dma_gather(transpose) of 512
                row-indices each (chunked: 1024 idxs in one call crashes).
    onehotPT[v,p] = (pos[b,p]==v)  (bf16)
    T = lhsT=Wt, rhs=onehotPT  -> T[p1,p2] = ALPHA*score[pos[p1],pos[p2]]
    out = s_arc[b] + T
"""

import sys

import numpy as np

_TRN_REPO = "/opt/trn_rl_repo"
if _TRN_REPO not in sys.path:
    sys.path.insert(0, _TRN_REPO)

import concourse.bass as bass  # noqa: E402
import concourse.mybir as mybir  # noqa: E402
from concourse import bacc  # noqa: E402
from concourse.tile import TileContext  # noqa: E402
from concourse.bass_utils import run_bass_kernel_spmd  # noqa: E402

N_POS = 1024
ALPHA = 0.3
B, S = 32, 1024
NCORES = 8
BPC = B // NCORES  # batches per core
KB = S // 128  # 8 k-blocks of 128
GCH = 512  # dma_gather idx chunk
F32 = mybir.dt.float32
F32R = mybir.dt.float32r
BF16 = mybir.dt.bfloat16
I16 = mybir.dt.int16

_CACHE = {}


def _build_nc():
    EQ = mybir.AluOpType.is_equal
    ADD = mybir.AluOpType.add
    nc = bacc.Bacc("TRN2", target_bir_lowering=False, debug=False, num_devices=NCORES)
    a_d = nc.dram_tensor("a", [BPC, S, S], F32R, kind="ExternalInput")
    s_d = nc.dram_tensor("s", [BPC, S, S], F32, kind="ExternalInput")
    adds_d = nc.dram_tensor("adds", [BPC, S], F32, kind="ExternalInput")
    pos_d = nc.dram_tensor("pos", [BPC, S], F32, kind="ExternalInput")
    pos16_d = nc.dram_tensor(
        "pos16", [BPC, S // GCH, 128, GCH // 16], I16, kind="ExternalInput"
    )
    o_d = nc.dram_tensor("o", [BPC, S, S], F32, kind="ExternalOutput")
    hist_in = [
        nc.dram_tensor(f"hist_in{h}", [N_POS // 2, N_POS], F32) for h in range(2)
    ]
    hist_out = [
        nc.dram_tensor(f"hist_out{h}", [N_POS // 2, N_POS], F32, addr_space="Shared")
        for h in range(2)
    ]
    score_bf = nc.dram_tensor("score_bf", [N_POS, N_POS], BF16)

    with TileContext(nc) as tc:
        with (
            tc.tile_pool(name="const", bufs=1) as pconst,
            tc.tile_pool(name="psum", bufs=8, space="PSUM") as pps,
        ):
            iota_v = pconst.tile([128, S], F32)
            nc.gpsimd.iota(
                iota_v[:],
                pattern=[[1, S]],
                base=0,
                channel_multiplier=0,
                allow_small_or_imprecise_dtypes=True,
            )
            iota2 = pconst.tile([128, KB], F32)
            nc.gpsimd.iota(
                iota2[:],
                pattern=[[128, KB]],
                base=0,
                channel_multiplier=1,
                allow_small_or_imprecise_dtypes=True,
            )

            # ---------------- Phase A: local histogram ----------------
            with (
                tc.tile_pool(name="pa", bufs=2) as pa,
                tc.tile_pool(name="poh", bufs=2) as poh,
                tc.tile_pool(name="pgt", bufs=1) as pgt,
                tc.tile_pool(name="phist", bufs=1) as phist,
                tc.tile_pool(name="pidx", bufs=2) as pidx,
            ):
                hist_sb = phist.tile([128, KB, N_POS], F32)
                for b in range(BPC):
                    adds_col = pidx.tile([128, KB], F32, tag="addsc")
                    nc.sync.dma_start(
                        adds_col[:], adds_d.ap()[b].rearrange("(mb q) -> q mb", q=128)
                    )
                    a_t = pa.tile([128, KB, S], F32R, tag="a")
                    nc.sync.dma_start(
                        a_t[:], a_d.ap()[b].rearrange("(kb kp) n -> kp kb n", kp=128)
                    )
                    oh = poh.tile([128, KB, N_POS], F32R, tag="oh")
                    for pb in range(KB):
                        nc.vector.tensor_scalar(
                            oh[:, pb, :], iota_v[:], adds_col[:, pb : pb + 1], None, op0=EQ
                        )
                    # MM1: Gt[p2, v1] = sum_p1 a[p1,p2] * onehotA[p1,v1]
                    gt = pgt.tile([128, KB, S], F32R, tag="gt")
                    for m in range(KB):
                        for n in range(2):
                            ps = pps.tile([128, 512], F32, tag="ps")
                            for k in range(KB):
                                nc.tensor.matmul(
                                    ps[:],
                                    a_t[:, k, m * 128 : (m + 1) * 128],
                                    oh[:, k, n * 512 : (n + 1) * 512],
                                    start=(k == 0),
                                    stop=(k == KB - 1),
                                )
                            nc.scalar.copy(gt[:, m, n * 512 : (n + 1) * 512], ps[:])
                    # MM2: hist2d[v1, v2] += sum_p2 Gt[p2,v1] * onehotA[p2,v2]
                    for m in range(KB):
                        for n in range(2):
                            ph = pps.tile([128, 512], F32, tag="ps")
                            for k in range(KB):
                                nc.tensor.matmul(
                                    ph[:],
                                    gt[:, k, m * 128 : (m + 1) * 128],
                                    oh[:, k, n * 512 : (n + 1) * 512],
                                    start=(k == 0),
                                    stop=(k == KB - 1),
                                )
                            dst = hist_sb[:, m, n * 512 : (n + 1) * 512]
                            if b == 0:
                                nc.vector.tensor_copy(dst, ph[:])
                            else:
                                nc.vector.tensor_tensor(dst, dst, ph[:], op=ADD)
                        # as soon as the last batch finishes a half of the
                        # histogram, ship it: DMA + AllReduce overlap the rest
                        if b == BPC - 1 and m in (KB // 2 - 1, KB - 1):
                            h = 0 if m < KB // 2 else 1
                            mlo = h * (KB // 2)
                            nc.sync.dma_start(
                                hist_in[h].ap().rearrange("(mb q) v -> q mb v", q=128),
                                hist_sb[:, mlo : mlo + KB // 2, :],
                            )
                            nc.gpsimd.collective_compute(
                                "AllReduce",
                                ADD,
                                replica_groups=[list(range(NCORES))],
                                ins=[hist_in[h][:]],
                                outs=[hist_out[h][:]],
                            )

            # ---------------- sigmoid + scale -> bf16 score table ----------
            with tc.tile_pool(name="praw", bufs=2) as praw:
                for h in range(2):
                    raw = praw.tile([128, KB // 2, N_POS], F32, tag="raw")
                    nc.sync.dma_start(
                        raw[:],
                        hist_out[h].ap().rearrange("(kb q) v -> q kb v", q=128),
                    )
                    sg = praw.tile([128, KB // 2, N_POS], F32, tag="sg")
                    nc.scalar.activation(
                        sg[:], raw[:], mybir.ActivationFunctionType.Sigmoid
                    )
                    sc = praw.tile([128, KB // 2, N_POS], BF16, tag="sc")
                    nc.vector.tensor_scalar_mul(sc[:], sg[:], ALPHA)
                    nc.sync.dma_start(
                        score_bf.ap()[h * 512 : (h + 1) * 512, :].rearrange(
                            "(kb q) v -> q kb v", q=128
                        ),
                        sc[:],
                    )

            # ---------------- Phase B: double gather + add ----------------
            with (
                tc.tile_pool(name="pohT", bufs=2) as pohT,
                tc.tile_pool(name="pwt", bufs=4) as pwt,
                tc.tile_pool(name="pst", bufs=2) as pst,
                tc.tile_pool(name="pposb", bufs=2) as pposb,
                tc.tile_pool(name="pg16", bufs=4) as pg16,
                tc.tile_pool(name="pout", bufs=4) as pout,
            ):
                for b in range(BPC):
                    # Wt[v2, p1] = score_bf[pos[p1], v2], p1 chunked by GCH
                    wt_c = []
                    for ci in range(S // GCH):
                        it = pg16.tile([128, GCH // 16], I16, tag="g16")
                        nc.sync.dma_start(it[:], pos16_d.ap()[b, ci])
                        wt = pwt.tile([128, KB, GCH], BF16, tag="wt")
                        nc.gpsimd.dma_gather(
                            out_ap=wt[:],
                            in_ap=score_bf.ap(),
                            idxs_ap=it[:],
                            num_idxs=GCH,
                            num_idxs_reg=GCH,
                            elem_size=N_POS,
                            transpose=True,
                        )
                        wt_c.append(wt)
                    pos_b = pposb.tile([128, S], F32, tag="posb")
                    nc.sync.dma_start(
                        pos_b[:],
                        pos_d.ap()[b].unsqueeze(0).partition_broadcast(128),
                    )
                    ohT = pohT.tile([128, KB, S], BF16, tag="ohT")
                    for c in range(KB):
                        nc.vector.tensor_scalar(
                            ohT[:, c, :], pos_b[:], iota2[:, c : c + 1], None, op0=EQ
                        )
                    s_t = pst.tile([128, KB, S], F32, tag="st")
                    nc.sync.dma_start(
                        s_t[:], s_d.ap()[b].rearrange("(mb q) n -> q mb n", q=128)
                    )
                    # MMb: T[p1, p2] = sum_v2 Wt[v2,p1] * onehotPT[v2,p2]
                    o_view = o_d.ap()[b].rearrange("(mb q) n -> q mb n", q=128)
                    for m in range(KB):
                        ci, off = m // (GCH // 128), (m % (GCH // 128)) * 128
                        for n in range(2):
                            pt = pps.tile([128, 512], F32, tag="ps")
                            for k in range(KB):
                                nc.tensor.matmul(
                                    pt[:],
                                    wt_c[ci][:, k, off : off + 128],
                                    ohT[:, k, n * 512 : (n + 1) * 512],
                                    start=(k == 0),
                                    stop=(k == KB - 1),
                                )
                            ot = pout.tile([128, 512], F32, tag="ot")
                            nc.vector.tensor_tensor(
                                ot[:],
                                s_t[:, m, n * 512 : (n + 1) * 512],
                                pt[:],
                                op=ADD,
                            )
                            nc.sync.dma_start(
                                o_view[:, m, n * 512 : (n + 1) * 512], ot[:]
                            )
    nc.finalize()
    return nc


def _get_nc():
    if "nc" not in _CACHE:
        _CACHE["nc"] = _build_nc()
    return _CACHE["nc"]


def _pos16(pos_shard):
    # wrapped-in-16-partitions layout per GCH-chunk, replicated across the
    # 8 gpsimd 16-partition groups
    out = np.empty((BPC, S // GCH, 128, GCH // 16), np.int16)
    for b in range(BPC):
        for ci in range(S // GCH):
            vals = pos_shard[b, ci * GCH : (ci + 1) * GCH].reshape(GCH // 16, 16)
            out[b, ci] = np.tile(vals.T, (8, 1))
    return out


def _make_in_maps(s_arc, a_arc, adds, pos):
    pos_i = np.asarray(pos).astype(np.int64)
    in_maps = []
    for c in range(NCORES):
        sl = slice(BPC * c, BPC * (c + 1))
        in_maps.append(
            {
                "a": np.ascontiguousarray(a_arc[sl], dtype=np.float32),
                "s": np.ascontiguousarray(s_arc[sl], dtype=np.float32),
                "adds": np.ascontiguousarray(adds[sl], dtype=np.float32),
                "pos": np.ascontiguousarray(pos[sl], dtype=np.float32),
                "pos16": _pos16(pos_i[sl]),
            }
        )
    return in_maps


def _run(in_maps, trace=False, **kwargs):
    return run_bass_kernel_spmd(
        _get_nc(), in_maps, core_ids=list(range(NCORES)), trace=trace, **kwargs
    )


def kernel(s_arc, a_arc, adds, pos):
    s_arc = np.asarray(s_arc)
    a_arc = np.asarray(a_arc)
    assert s_arc.shape == (B, S, S) and a_arc.shape == (B, S, S), (
        s_arc.shape,
        a_arc.shape,
    )
    res = _run(_make_in_maps(s_arc, a_arc, adds, pos), trace=False)
    out = np.empty((B, S, S), dtype=np.float32)
    for c in range(NCORES):
        out[BPC * c : BPC * (c + 1)] = res.results[c]["o"]
    return out


# revision 7
# speedup vs baseline: 1.2394x; 1.1098x over previous
"""Trainium2 Bass kernel for nn_EnsembleModel histogram_binning.

Math (reference):
    pair_idx[b,p1,p2] = adds[b,p1]*1024 + adds[b,p2]
    hist = segment_sum(a_arc, pair_idx, 1024*1024)       # scatter-add
    score = sigmoid(hist)
    out[b,p1,p2] = s_arc[b,p1,p2] + 0.3*score[pos[b,p1]*1024 + pos[b,p2]]

Device formulation, data-parallel over batch (core c owns batches [4c,4c+4)):
  Phase A (per local batch, fp32r matmuls on the PE):
    onehotA[p,v] = (adds[b,p]==v)              [1024,1024]
    Gt   = a_arc[b].T @ onehotA                (lhsT=a_arc -> out[p2,v1])
    hist2d += Gt.T @ onehotA                   (lhsT=Gt    -> out[v1,v2])
  AllReduce(hist2d) over 8 cores, split in two 512-row halves so the first
  half reduces while the second is still being computed.
  score_bf = bf16(ALPHA * sigmoid(hist2d)) staged to DRAM.
  Phase B (per local batch):
    Wt[v2,p1] = score_bf[pos[p1], v2]   via 2x dma_gather(transpose) of 512
                row-indices each (chunked: 1024 idxs in one call crashes).
    onehotPT[v,p] = (pos[b,p]==v)  (bf16)
    T = lhsT=Wt, rhs=onehotPT  -> T[p1,p2] = ALPHA*score[pos[p1],pos[p2]]
    out = s_arc[b] + T
"""

import sys

import numpy as np
import ml_dtypes

_bf16 = ml_dtypes.bfloat16

_TRN_REPO = "/opt/trn_rl_repo"
if _TRN_REPO not in sys.path:
    sys.path.insert(0, _TRN_REPO)

import concourse.bass as bass  # noqa: E402
import concourse.mybir as mybir  # noqa: E402
from concourse import bacc  # noqa: E402
from concourse.tile import TileContext  # noqa: E402
from concourse.bass_utils import run_bass_kernel_spmd  # noqa: E402

N_POS = 1024
ALPHA = 0.3
B, S = 32, 1024
NCORES = 8
BPC = B // NCORES  # batches per core
KB = S // 128  # 8 k-blocks of 128
GCH = 512  # dma_gather idx chunk
F32 = mybir.dt.float32
F32R = mybir.dt.float32r
BF16 = mybir.dt.bfloat16
I16 = mybir.dt.int16

_CACHE = {}


def _build_nc():
    EQ = mybir.AluOpType.is_equal
    ADD = mybir.AluOpType.add
    nc = bacc.Bacc("TRN2", target_bir_lowering=False, debug=False, num_devices=NCORES)
    a_d = nc.dram_tensor("a", [BPC, S, S], BF16, kind="ExternalInput")
    s_d = nc.dram_tensor("s", [BPC, S, S], F32, kind="ExternalInput")
    adds_d = nc.dram_tensor("adds", [BPC, S], F32, kind="ExternalInput")
    pos_d = nc.dram_tensor("pos", [BPC, S], F32, kind="ExternalInput")
    pos16_d = nc.dram_tensor(
        "pos16", [BPC, S // GCH, 128, GCH // 16], I16, kind="ExternalInput"
    )
    o_d = nc.dram_tensor("o", [BPC, S, S], F32, kind="ExternalOutput")
    hist_in = [
        nc.dram_tensor(f"hist_in{h}", [N_POS // 2, N_POS], F32) for h in range(2)
    ]
    hist_out = [
        nc.dram_tensor(f"hist_out{h}", [N_POS // 2, N_POS], F32, addr_space="Shared")
        for h in range(2)
    ]
    score_bf = nc.dram_tensor("score_bf", [N_POS, N_POS], BF16)

    with TileContext(nc) as tc:
        with (
            tc.tile_pool(name="const", bufs=1) as pconst,
            tc.tile_pool(name="psum", bufs=8, space="PSUM") as pps,
        ):
            iota_v = pconst.tile([128, S], F32)
            nc.gpsimd.iota(
                iota_v[:],
                pattern=[[1, S]],
                base=0,
                channel_multiplier=0,
                allow_small_or_imprecise_dtypes=True,
            )
            iota2 = pconst.tile([128, KB], F32)
            nc.gpsimd.iota(
                iota2[:],
                pattern=[[128, KB]],
                base=0,
                channel_multiplier=1,
                allow_small_or_imprecise_dtypes=True,
            )

            # ---------------- Phase A: local histogram ----------------
            with (
                tc.tile_pool(name="pa", bufs=2) as pa,
                tc.tile_pool(name="poh", bufs=4) as poh,
                tc.tile_pool(name="pgt", bufs=4) as pgt,
                tc.tile_pool(name="phist", bufs=1) as phist,
                tc.tile_pool(name="pidx", bufs=2) as pidx,
            ):
                hist_sb = phist.tile([128, KB, N_POS], F32)
                ohs, gts = [], []
                for b in range(BPC):
                    adds_col = pidx.tile([128, KB], F32, tag="addsc")
                    nc.sync.dma_start(
                        adds_col[:], adds_d.ap()[b].rearrange("(mb q) -> q mb", q=128)
                    )
                    a_t = pa.tile([128, KB, S], BF16, tag="a")
                    nc.sync.dma_start(
                        a_t[:], a_d.ap()[b].rearrange("(kb kp) n -> kp kb n", kp=128)
                    )
                    oh = poh.tile([128, KB, N_POS], BF16, tag="oh")
                    for pb in range(KB):
                        nc.vector.tensor_scalar(
                            oh[:, pb, :], iota_v[:], adds_col[:, pb : pb + 1], None, op0=EQ
                        )
                    ohs.append(oh)
                    # MM1: Gt[p2, v1] = sum_p1 a[p1,p2] * onehotA[p1,v1]
                    gt = pgt.tile([128, KB, S], BF16, tag="gt")
                    for m in range(KB):
                        for n in range(2):
                            ps = pps.tile([128, 512], F32, tag="ps")
                            for k in range(KB):
                                nc.tensor.matmul(
                                    ps[:],
                                    a_t[:, k, m * 128 : (m + 1) * 128],
                                    oh[:, k, n * 512 : (n + 1) * 512],
                                    start=(k == 0),
                                    stop=(k == KB - 1),
                                )
                            nc.vector.tensor_copy(gt[:, m, n * 512 : (n + 1) * 512], ps[:])
                    gts.append(gt)
                # MM2: hist2d[v1, v2] = sum_{b,p2} Gt[p2,v1] * onehotA[p2,v2]
                # slice-by-slice over v1 so the first half AllReduces while
                # the second half is still on the PE
                for m in range(KB):
                    for n in range(2):
                        ph = pps.tile([128, 512], F32, tag="ps")
                        for b in range(BPC):
                            for k in range(KB):
                                nc.tensor.matmul(
                                    ph[:],
                                    gts[b][:, k, m * 128 : (m + 1) * 128],
                                    ohs[b][:, k, n * 512 : (n + 1) * 512],
                                    start=(b == 0 and k == 0),
                                    stop=(b == BPC - 1 and k == KB - 1),
                                )
                        nc.vector.tensor_copy(
                            hist_sb[:, m, n * 512 : (n + 1) * 512], ph[:]
                        )
                    if m in (KB // 2 - 1, KB - 1):
                        h = 0 if m < KB // 2 else 1
                        mlo = h * (KB // 2)
                        nc.sync.dma_start(
                            hist_in[h].ap().rearrange("(mb q) v -> q mb v", q=128),
                            hist_sb[:, mlo : mlo + KB // 2, :],
                        )
                        nc.gpsimd.collective_compute(
                            "AllReduce",
                            ADD,
                            replica_groups=[list(range(NCORES))],
                            ins=[hist_in[h][:]],
                            outs=[hist_out[h][:]],
                        )

            # ---------------- sigmoid + scale -> bf16 score table ----------
            with tc.tile_pool(name="praw", bufs=2) as praw:
                for h in range(2):
                    raw = praw.tile([128, KB // 2, N_POS], F32, tag="raw")
                    nc.sync.dma_start(
                        raw[:],
                        hist_out[h].ap().rearrange("(kb q) v -> q kb v", q=128),
                    )
                    sg = praw.tile([128, KB // 2, N_POS], F32, tag="sg")
                    nc.scalar.activation(
                        sg[:], raw[:], mybir.ActivationFunctionType.Sigmoid
                    )
                    sc = praw.tile([128, KB // 2, N_POS], BF16, tag="sc")
                    nc.vector.tensor_scalar_mul(sc[:], sg[:], ALPHA)
                    nc.sync.dma_start(
                        score_bf.ap()[h * 512 : (h + 1) * 512, :].rearrange(
                            "(kb q) v -> q kb v", q=128
                        ),
                        sc[:],
                    )

            # ---------------- Phase B: double gather + add ----------------
            with (
                tc.tile_pool(name="pohT", bufs=2) as pohT,
                tc.tile_pool(name="pwt", bufs=4) as pwt,
                tc.tile_pool(name="pst", bufs=2) as pst,
                tc.tile_pool(name="pposb", bufs=2) as pposb,
                tc.tile_pool(name="pg16", bufs=4) as pg16,
                tc.tile_pool(name="pout", bufs=4) as pout,
            ):
                for b in range(BPC):
                    # Wt[v2, p1] = score_bf[pos[p1], v2], p1 chunked by GCH
                    wt_c = []
                    for ci in range(S // GCH):
                        it = pg16.tile([128, GCH // 16], I16, tag="g16")
                        nc.sync.dma_start(it[:], pos16_d.ap()[b, ci])
                        wt = pwt.tile([128, KB, GCH], BF16, tag="wt")
                        nc.gpsimd.dma_gather(
                            out_ap=wt[:],
                            in_ap=score_bf.ap(),
                            idxs_ap=it[:],
                            num_idxs=GCH,
                            num_idxs_reg=GCH,
                            elem_size=N_POS,
                            transpose=True,
                        )
                        wt_c.append(wt)
                    pos_b = pposb.tile([128, S], F32, tag="posb")
                    nc.sync.dma_start(
                        pos_b[:],
                        pos_d.ap()[b].unsqueeze(0).partition_broadcast(128),
                    )
                    ohT = pohT.tile([128, KB, S], BF16, tag="ohT")
                    for c in range(KB):
                        nc.vector.tensor_scalar(
                            ohT[:, c, :], pos_b[:], iota2[:, c : c + 1], None, op0=EQ
                        )
                    s_t = pst.tile([128, KB, S], F32, tag="st")
                    nc.sync.dma_start(
                        s_t[:], s_d.ap()[b].rearrange("(mb q) n -> q mb n", q=128)
                    )
                    # MMb: T[p1, p2] = sum_v2 Wt[v2,p1] * onehotPT[v2,p2]
                    o_view = o_d.ap()[b].rearrange("(mb q) n -> q mb n", q=128)
                    for m in range(KB):
                        ci, off = m // (GCH // 128), (m % (GCH // 128)) * 128
                        for n in range(2):
                            pt = pps.tile([128, 512], F32, tag="ps")
                            for k in range(KB):
                                nc.tensor.matmul(
                                    pt[:],
                                    wt_c[ci][:, k, off : off + 128],
                                    ohT[:, k, n * 512 : (n + 1) * 512],
                                    start=(k == 0),
                                    stop=(k == KB - 1),
                                )
                            ot = pout.tile([128, 512], F32, tag="ot")
                            nc.vector.tensor_tensor(
                                ot[:],
                                s_t[:, m, n * 512 : (n + 1) * 512],
                                pt[:],
                                op=ADD,
                            )
                            nc.sync.dma_start(
                                o_view[:, m, n * 512 : (n + 1) * 512], ot[:]
                            )
    nc.finalize()
    return nc


def _get_nc():
    if "nc" not in _CACHE:
        _CACHE["nc"] = _build_nc()
    return _CACHE["nc"]


def _pos16(pos_shard):
    # wrapped-in-16-partitions layout per GCH-chunk, replicated across the
    # 8 gpsimd 16-partition groups
    out = np.empty((BPC, S // GCH, 128, GCH // 16), np.int16)
    for b in range(BPC):
        for ci in range(S // GCH):
            vals = pos_shard[b, ci * GCH : (ci + 1) * GCH].reshape(GCH // 16, 16)
            out[b, ci] = np.tile(vals.T, (8, 1))
    return out


def _make_in_maps(s_arc, a_arc, adds, pos):
    pos_i = np.asarray(pos).astype(np.int64)
    in_maps = []
    for c in range(NCORES):
        sl = slice(BPC * c, BPC * (c + 1))
        in_maps.append(
            {
                "a": np.ascontiguousarray(a_arc[sl]).astype(_bf16),
                "s": np.ascontiguousarray(s_arc[sl], dtype=np.float32),
                "adds": np.ascontiguousarray(adds[sl], dtype=np.float32),
                "pos": np.ascontiguousarray(pos[sl], dtype=np.float32),
                "pos16": _pos16(pos_i[sl]),
            }
        )
    return in_maps


def _run(in_maps, trace=False, **kwargs):
    return run_bass_kernel_spmd(
        _get_nc(), in_maps, core_ids=list(range(NCORES)), trace=trace, **kwargs
    )


def kernel(s_arc, a_arc, adds, pos):
    s_arc = np.asarray(s_arc)
    a_arc = np.asarray(a_arc)
    assert s_arc.shape == (B, S, S) and a_arc.shape == (B, S, S), (
        s_arc.shape,
        a_arc.shape,
    )
    res = _run(_make_in_maps(s_arc, a_arc, adds, pos), trace=False)
    out = np.empty((B, S, S), dtype=np.float32)
    for c in range(NCORES):
        out[BPC * c : BPC * (c + 1)] = res.results[c]["o"]
    return out


# revision 11
# speedup vs baseline: 1.2576x; 1.0147x over previous
"""Trainium2 Bass kernel for nn_EnsembleModel histogram_binning.

Math (reference):
    pair_idx[b,p1,p2] = adds[b,p1]*1024 + adds[b,p2]
    hist = segment_sum(a_arc, pair_idx, 1024*1024)       # scatter-add
    score = sigmoid(hist)
    out[b,p1,p2] = s_arc[b,p1,p2] + 0.3*score[pos[b,p1]*1024 + pos[b,p2]]

Device formulation, data-parallel over batch (core c owns batches [4c,4c+4)):
  Phase A (per local batch, fp32r matmuls on the PE):
    onehotA[p,v] = (adds[b,p]==v)              [1024,1024]
    Gt   = a_arc[b].T @ onehotA                (lhsT=a_arc -> out[p2,v1])
    hist2d += Gt.T @ onehotA                   (lhsT=Gt    -> out[v1,v2])
  AllReduce(hist2d) over 8 cores, split in two 512-row halves so the first
  half reduces while the second is still being computed.
  score_bf = bf16(ALPHA * sigmoid(hist2d)) staged to DRAM.
  Phase B (per local batch):
    Wt[v2,p1] = score_bf[pos[p1], v2]   via 2x dma_gather(transpose) of 512
                row-indices each (chunked: 1024 idxs in one call crashes).
    onehotPT[v,p] = (pos[b,p]==v)  (bf16)
    T = lhsT=Wt, rhs=onehotPT  -> T[p1,p2] = ALPHA*score[pos[p1],pos[p2]]
    out = s_arc[b] + T
"""

import sys

import numpy as np
import ml_dtypes

_bf16 = ml_dtypes.bfloat16

_TRN_REPO = "/opt/trn_rl_repo"
if _TRN_REPO not in sys.path:
    sys.path.insert(0, _TRN_REPO)

import concourse.bass as bass  # noqa: E402
import concourse.mybir as mybir  # noqa: E402
from concourse import bacc  # noqa: E402
from concourse.tile import TileContext  # noqa: E402
from concourse.bass_utils import run_bass_kernel_spmd  # noqa: E402

N_POS = 1024
ALPHA = 0.3
B, S = 32, 1024
NCORES = 8
BPC = B // NCORES  # batches per core
KB = S // 128  # 8 k-blocks of 128
GCH = 512  # dma_gather idx chunk
F32 = mybir.dt.float32
F32R = mybir.dt.float32r
BF16 = mybir.dt.bfloat16
I16 = mybir.dt.int16

_CACHE = {}


def _build_nc():
    EQ = mybir.AluOpType.is_equal
    ADD = mybir.AluOpType.add
    nc = bacc.Bacc("TRN2", target_bir_lowering=False, debug=False, num_devices=NCORES)
    a_d = nc.dram_tensor("a", [BPC, S, S], BF16, kind="ExternalInput")
    s_d = nc.dram_tensor("s", [BPC, S, S], F32, kind="ExternalInput")
    adds_d = nc.dram_tensor("adds", [BPC, S], F32, kind="ExternalInput")
    pos_d = nc.dram_tensor("pos", [BPC, S], F32, kind="ExternalInput")
    pos16_d = nc.dram_tensor(
        "pos16", [BPC, S // GCH, 128, GCH // 16], I16, kind="ExternalInput"
    )
    o_d = nc.dram_tensor("o", [BPC, S, S], F32, kind="ExternalOutput")
    hist_in = [
        nc.dram_tensor(f"hist_in{h}", [N_POS // 2, N_POS], F32) for h in range(2)
    ]
    hist_out = [
        nc.dram_tensor(f"hist_out{h}", [N_POS // 2, N_POS], F32, addr_space="Shared")
        for h in range(2)
    ]
    score_bf = nc.dram_tensor("score_bf", [N_POS, N_POS], BF16)

    with TileContext(nc) as tc:
        with (
            tc.tile_pool(name="const", bufs=1) as pconst,
            tc.tile_pool(name="psum", bufs=8, space="PSUM") as pps,
        ):
            iota_v = pconst.tile([128, S], F32)
            nc.gpsimd.iota(
                iota_v[:],
                pattern=[[1, S]],
                base=0,
                channel_multiplier=0,
                allow_small_or_imprecise_dtypes=True,
            )
            iota2 = pconst.tile([128, KB], F32)
            nc.gpsimd.iota(
                iota2[:],
                pattern=[[128, KB]],
                base=0,
                channel_multiplier=1,
                allow_small_or_imprecise_dtypes=True,
            )

            # ---------------- Phase A: local histogram ----------------
            with (
                tc.tile_pool(name="pa", bufs=2) as pa,
                tc.tile_pool(name="poh", bufs=4) as poh,
                tc.tile_pool(name="pgt", bufs=4) as pgt,
                tc.tile_pool(name="phist", bufs=1) as phist,
                tc.tile_pool(name="pidx", bufs=2) as pidx,
            ):
                hist_sb = phist.tile([128, KB, N_POS], F32)
                ohs, gts = [], []
                for b in range(BPC):
                    adds_col = pidx.tile([128, KB], F32, tag="addsc")
                    nc.gpsimd.dma_start(
                        adds_col[:], adds_d.ap()[b].rearrange("(mb q) -> q mb", q=128)
                    )
                    a_t = pa.tile([128, KB, S], BF16, tag="a")
                    nc.sync.dma_start(
                        a_t[:], a_d.ap()[b].rearrange("(kb kp) n -> kp kb n", kp=128)
                    )
                    oh = poh.tile([128, KB, N_POS], BF16, tag="oh")
                    for pb in range(KB):
                        nc.vector.tensor_scalar(
                            oh[:, pb, :], iota_v[:], adds_col[:, pb : pb + 1], None, op0=EQ
                        )
                    ohs.append(oh)
                    # MM1: Gt[p2, v1] = sum_p1 a[p1,p2] * onehotA[p1,v1]
                    gt = pgt.tile([128, KB, S], BF16, tag="gt")
                    for m in range(KB):
                        for n in range(2):
                            ps = pps.tile([128, 512], F32, tag="ps")
                            for k in range(KB):
                                nc.tensor.matmul(
                                    ps[:],
                                    a_t[:, k, m * 128 : (m + 1) * 128],
                                    oh[:, k, n * 512 : (n + 1) * 512],
                                    start=(k == 0),
                                    stop=(k == KB - 1),
                                )
                            nc.vector.tensor_copy(gt[:, m, n * 512 : (n + 1) * 512], ps[:])
                    gts.append(gt)
                # MM2: hist2d[v1, v2] = sum_{b,p2} Gt[p2,v1] * onehotA[p2,v2]
                # slice-by-slice over v1 so the first half AllReduces while
                # the second half is still on the PE
                for m in range(KB):
                    for n in range(2):
                        ph = pps.tile([128, 512], F32, tag="ps")
                        for b in range(BPC):
                            for k in range(KB):
                                nc.tensor.matmul(
                                    ph[:],
                                    gts[b][:, k, m * 128 : (m + 1) * 128],
                                    ohs[b][:, k, n * 512 : (n + 1) * 512],
                                    start=(b == 0 and k == 0),
                                    stop=(b == BPC - 1 and k == KB - 1),
                                )
                        nc.vector.tensor_copy(
                            hist_sb[:, m, n * 512 : (n + 1) * 512], ph[:]
                        )
                    if m in (KB // 2 - 1, KB - 1):
                        h = 0 if m < KB // 2 else 1
                        mlo = h * (KB // 2)
                        nc.scalar.dma_start(
                            hist_in[h].ap().rearrange("(mb q) v -> q mb v", q=128),
                            hist_sb[:, mlo : mlo + KB // 2, :],
                        )
                        nc.gpsimd.collective_compute(
                            "AllReduce",
                            ADD,
                            replica_groups=[list(range(NCORES))],
                            ins=[hist_in[h][:]],
                            outs=[hist_out[h][:]],
                        )

            # ---------- phase B prelude (score-independent) + score pipeline ----
            with (
                tc.tile_pool(name="pohT", bufs=4) as pohT,
                tc.tile_pool(name="pwt", bufs=4) as pwt,
                tc.tile_pool(name="pst", bufs=3) as pst,
                tc.tile_pool(name="pposb", bufs=2) as pposb,
                tc.tile_pool(name="pg16", bufs=8) as pg16,
                tc.tile_pool(name="pout", bufs=4) as pout,
                tc.tile_pool(name="praw", bufs=2) as praw,
            ):
                # prelude: one-hots + gather indices for all batches
                ohTs, its = [], []
                for b in range(BPC):
                    pos_b = pposb.tile([128, S], F32, tag="posb")
                    nc.gpsimd.dma_start(
                        pos_b[:],
                        pos_d.ap()[b].unsqueeze(0).partition_broadcast(128),
                    )
                    ohT = pohT.tile([128, KB, S], BF16, tag="ohT")
                    for c in range(KB):
                        nc.vector.tensor_scalar(
                            ohT[:, c, :], pos_b[:], iota2[:, c : c + 1], None, op0=EQ
                        )
                    ohTs.append(ohT)
                    for ci in range(S // GCH):
                        it = pg16.tile([128, GCH // 16], I16, tag="g16")
                        nc.gpsimd.dma_start(it[:], pos16_d.ap()[b, ci])
                        its.append(it)

                # sigmoid + ALPHA scale in quarter slices, scalar-engine queue
                QS = KB // 4  # 2 v1-blocks per quarter
                for qi in range(4):
                    h, qh = qi // 2, qi % 2
                    raw = praw.tile([128, QS, N_POS], F32, tag="raw")
                    nc.scalar.dma_start(
                        raw[:],
                        hist_out[h]
                        .ap()[qh * 256 : (qh + 1) * 256, :]
                        .rearrange("(kb q) v -> q kb v", q=128),
                    )
                    nc.scalar.activation(
                        raw[:], raw[:], mybir.ActivationFunctionType.Sigmoid
                    )
                    sc = praw.tile([128, QS, N_POS], BF16, tag="sc")
                    nc.vector.tensor_scalar_mul(sc[:], raw[:], ALPHA)
                    nc.scalar.dma_start(
                        score_bf.ap()[qi * 256 : (qi + 1) * 256, :].rearrange(
                            "(kb q) v -> q kb v", q=128
                        ),
                        sc[:],
                    )

                # all gathers up front (gpsimd queue), then the matmul stream
                wt_cs = []
                for b in range(BPC):
                    for ci in range(S // GCH):
                        wt = pwt.tile([128, KB, GCH], BF16, tag="wt")
                        nc.gpsimd.dma_gather(
                            out_ap=wt[:],
                            in_ap=score_bf.ap(),
                            idxs_ap=its[b * (S // GCH) + ci][:],
                            num_idxs=GCH,
                            num_idxs_reg=GCH,
                            elem_size=N_POS,
                            transpose=True,
                        )
                        wt_cs.append(wt)

                for b in range(BPC):
                    o_view = o_d.ap()[b].rearrange("(mb q) n -> q mb n", q=128)
                    s_view = s_d.ap()[b].rearrange("(mb q) n -> q mb n", q=128)
                    for half in range(2):
                        mlo = half * (KB // 2)
                        s_t = pst.tile([128, KB // 2, S], F32, tag="st")
                        nc.scalar.dma_start(
                            s_t[:], s_view[:, mlo : mlo + KB // 2, :]
                        )
                        # MMb: T[p1, p2] = sum_v2 Wt[v2,p1] * onehotPT[v2,p2]
                        for m in range(mlo, mlo + KB // 2):
                            ci, off = m // (GCH // 128), (m % (GCH // 128)) * 128
                            wt = wt_cs[b * (S // GCH) + ci]
                            for n in range(2):
                                pt = pps.tile([128, 512], F32, tag="ps")
                                for k in range(KB):
                                    nc.tensor.matmul(
                                        pt[:],
                                        wt[:, k, off : off + 128],
                                        ohTs[b][:, k, n * 512 : (n + 1) * 512],
                                        start=(k == 0),
                                        stop=(k == KB - 1),
                                    )
                                ot = pout.tile([128, 512], F32, tag="ot")
                                nc.vector.tensor_tensor(
                                    ot[:],
                                    s_t[:, m - mlo, n * 512 : (n + 1) * 512],
                                    pt[:],
                                    op=ADD,
                                )
                                nc.sync.dma_start(
                                    o_view[:, m, n * 512 : (n + 1) * 512], ot[:]
                                )
    nc.finalize()
    return nc


def _get_nc():
    if "nc" not in _CACHE:
        _CACHE["nc"] = _build_nc()
    return _CACHE["nc"]


def _pos16(pos_shard):
    # wrapped-in-16-partitions layout per GCH-chunk, replicated across the
    # 8 gpsimd 16-partition groups
    out = np.empty((BPC, S // GCH, 128, GCH // 16), np.int16)
    for b in range(BPC):
        for ci in range(S // GCH):
            vals = pos_shard[b, ci * GCH : (ci + 1) * GCH].reshape(GCH // 16, 16)
            out[b, ci] = np.tile(vals.T, (8, 1))
    return out


def _make_in_maps(s_arc, a_arc, adds, pos):
    pos_i = np.asarray(pos).astype(np.int64)
    in_maps = []
    for c in range(NCORES):
        sl = slice(BPC * c, BPC * (c + 1))
        in_maps.append(
            {
                "a": np.ascontiguousarray(a_arc[sl]).astype(_bf16),
                "s": np.ascontiguousarray(s_arc[sl], dtype=np.float32),
                "adds": np.ascontiguousarray(adds[sl], dtype=np.float32),
                "pos": np.ascontiguousarray(pos[sl], dtype=np.float32),
                "pos16": _pos16(pos_i[sl]),
            }
        )
    return in_maps


def _run(in_maps, trace=False, **kwargs):
    return run_bass_kernel_spmd(
        _get_nc(), in_maps, core_ids=list(range(NCORES)), trace=trace, **kwargs
    )


def kernel(s_arc, a_arc, adds, pos):
    s_arc = np.asarray(s_arc)
    a_arc = np.asarray(a_arc)
    assert s_arc.shape == (B, S, S) and a_arc.shape == (B, S, S), (
        s_arc.shape,
        a_arc.shape,
    )
    in_maps = _make_in_maps(s_arc, a_arc, adds, pos)
    try:
        res = _run(in_maps, trace=False)
    except Exception:
        res = _run(in_maps, trace=False)
    out = np.empty((B, S, S), dtype=np.float32)
    for c in range(NCORES):
        out[BPC * c : BPC * (c + 1)] = res.results[c]["o"]
    return out


# revision 12
# speedup vs baseline: 1.2627x; 1.0040x over previous
"""Trainium2 Bass kernel for nn_EnsembleModel histogram_binning.

Math (reference):
    pair_idx[b,p1,p2] = adds[b,p1]*1024 + adds[b,p2]
    hist = segment_sum(a_arc, pair_idx, 1024*1024)       # scatter-add
    score = sigmoid(hist)
    out[b,p1,p2] = s_arc[b,p1,p2] + 0.3*score[pos[b,p1]*1024 + pos[b,p2]]

Device formulation, data-parallel over batch (core c owns batches [4c,4c+4)):
  Phase A (per local batch, fp32r matmuls on the PE):
    onehotA[p,v] = (adds[b,p]==v)              [1024,1024]
    Gt   = a_arc[b].T @ onehotA                (lhsT=a_arc -> out[p2,v1])
    hist2d += Gt.T @ onehotA                   (lhsT=Gt    -> out[v1,v2])
  AllReduce(hist2d) over 8 cores, split in two 512-row halves so the first
  half reduces while the second is still being computed.
  score_bf = bf16(ALPHA * sigmoid(hist2d)) staged to DRAM.
  Phase B (per local batch):
    Wt[v2,p1] = score_bf[pos[p1], v2]   via 2x dma_gather(transpose) of 512
                row-indices each (chunked: 1024 idxs in one call crashes).
    onehotPT[v,p] = (pos[b,p]==v)  (bf16)
    T = lhsT=Wt, rhs=onehotPT  -> T[p1,p2] = ALPHA*score[pos[p1],pos[p2]]
    out = s_arc[b] + T
"""

import sys

import numpy as np
import ml_dtypes

_bf16 = ml_dtypes.bfloat16

_TRN_REPO = "/opt/trn_rl_repo"
if _TRN_REPO not in sys.path:
    sys.path.insert(0, _TRN_REPO)

import concourse.bass as bass  # noqa: E402
import concourse.mybir as mybir  # noqa: E402
from concourse import bacc  # noqa: E402
from concourse.tile import TileContext  # noqa: E402
from concourse.bass_utils import run_bass_kernel_spmd  # noqa: E402

N_POS = 1024
ALPHA = 0.3
B, S = 32, 1024
NCORES = 8
BPC = B // NCORES  # batches per core
KB = S // 128  # 8 k-blocks of 128
GCH = 512  # dma_gather idx chunk
F32 = mybir.dt.float32
F32R = mybir.dt.float32r
BF16 = mybir.dt.bfloat16
I16 = mybir.dt.int16

_CACHE = {}


def _build_nc():
    EQ = mybir.AluOpType.is_equal
    ADD = mybir.AluOpType.add
    nc = bacc.Bacc("TRN2", target_bir_lowering=False, debug=False, num_devices=NCORES)
    a_d = nc.dram_tensor("a", [BPC, S, S], BF16, kind="ExternalInput")
    s_d = nc.dram_tensor("s", [BPC, S, S], F32, kind="ExternalInput")
    adds_d = nc.dram_tensor("adds", [BPC, S], F32, kind="ExternalInput")
    pos_d = nc.dram_tensor("pos", [BPC, S], F32, kind="ExternalInput")
    pos16_d = nc.dram_tensor(
        "pos16", [BPC, S // GCH, 128, GCH // 16], I16, kind="ExternalInput"
    )
    o_d = nc.dram_tensor("o", [BPC, S, S], F32, kind="ExternalOutput")
    hist_in = [
        nc.dram_tensor(f"hist_in{h}", [N_POS // 2, N_POS], F32) for h in range(2)
    ]
    hist_out = [
        nc.dram_tensor(f"hist_out{h}", [N_POS // 2, N_POS], F32, addr_space="Shared")
        for h in range(2)
    ]
    score_bf = nc.dram_tensor("score_bf", [N_POS, N_POS], BF16)

    with TileContext(nc) as tc:
        with (
            tc.tile_pool(name="const", bufs=1) as pconst,
            tc.tile_pool(name="psum", bufs=8, space="PSUM") as pps,
        ):
            iota_v = pconst.tile([128, S], F32)
            nc.gpsimd.iota(
                iota_v[:],
                pattern=[[1, S]],
                base=0,
                channel_multiplier=0,
                allow_small_or_imprecise_dtypes=True,
            )
            iota2 = pconst.tile([128, KB], F32)
            nc.gpsimd.iota(
                iota2[:],
                pattern=[[128, KB]],
                base=0,
                channel_multiplier=1,
                allow_small_or_imprecise_dtypes=True,
            )

            # ---------------- Phase A: local histogram ----------------
            with (
                tc.tile_pool(name="pa", bufs=2) as pa,
                tc.tile_pool(name="poh", bufs=4) as poh,
                tc.tile_pool(name="pgt", bufs=4) as pgt,
                tc.tile_pool(name="phist", bufs=1) as phist,
                tc.tile_pool(name="pidx", bufs=2) as pidx,
            ):
                hist_sb = phist.tile([128, KB, N_POS], F32)
                ohs, gts = [], []
                for b in range(BPC):
                    adds_col = pidx.tile([128, KB], F32, tag="addsc")
                    nc.gpsimd.dma_start(
                        adds_col[:], adds_d.ap()[b].rearrange("(mb q) -> q mb", q=128)
                    )
                    a_t = pa.tile([128, KB, S], BF16, tag="a")
                    nc.sync.dma_start(
                        a_t[:], a_d.ap()[b].rearrange("(kb kp) n -> kp kb n", kp=128)
                    )
                    oh = poh.tile([128, KB, N_POS], BF16, tag="oh")
                    for pb in range(KB):
                        nc.vector.tensor_scalar(
                            oh[:, pb, :], iota_v[:], adds_col[:, pb : pb + 1], None, op0=EQ
                        )
                    ohs.append(oh)
                    # MM1: Gt[p2, v1] = sum_p1 a[p1,p2] * onehotA[p1,v1]
                    gt = pgt.tile([128, KB, S], BF16, tag="gt")
                    for m in range(KB):
                        for n in range(2):
                            ps = pps.tile([128, 512], F32, tag="ps")
                            for k in range(KB):
                                nc.tensor.matmul(
                                    ps[:],
                                    a_t[:, k, m * 128 : (m + 1) * 128],
                                    oh[:, k, n * 512 : (n + 1) * 512],
                                    start=(k == 0),
                                    stop=(k == KB - 1),
                                )
                            nc.vector.tensor_copy(gt[:, m, n * 512 : (n + 1) * 512], ps[:])
                    gts.append(gt)
                # MM2: hist2d[v1, v2] = sum_{b,p2} Gt[p2,v1] * onehotA[p2,v2]
                # slice-by-slice over v1 so the first half AllReduces while
                # the second half is still on the PE
                for m in range(KB):
                    for n in range(2):
                        ph = pps.tile([128, 512], F32, tag="ps")
                        for b in range(BPC):
                            for k in range(KB):
                                nc.tensor.matmul(
                                    ph[:],
                                    gts[b][:, k, m * 128 : (m + 1) * 128],
                                    ohs[b][:, k, n * 512 : (n + 1) * 512],
                                    start=(b == 0 and k == 0),
                                    stop=(b == BPC - 1 and k == KB - 1),
                                )
                        nc.vector.tensor_copy(
                            hist_sb[:, m, n * 512 : (n + 1) * 512], ph[:]
                        )
                    if m in (KB // 2 - 1, KB - 1):
                        h = 0 if m < KB // 2 else 1
                        mlo = h * (KB // 2)
                        nc.scalar.dma_start(
                            hist_in[h].ap().rearrange("(mb q) v -> q mb v", q=128),
                            hist_sb[:, mlo : mlo + KB // 2, :],
                        )
                        nc.gpsimd.collective_compute(
                            "AllReduce",
                            ADD,
                            replica_groups=[list(range(NCORES))],
                            ins=[hist_in[h][:]],
                            outs=[hist_out[h][:]],
                        )

            # ---------- phase B prelude (score-independent) + score pipeline ----
            with (
                tc.tile_pool(name="pohT", bufs=4) as pohT,
                tc.tile_pool(name="pwt", bufs=4) as pwt,
                tc.tile_pool(name="pst", bufs=3) as pst,
                tc.tile_pool(name="pposb", bufs=2) as pposb,
                tc.tile_pool(name="pg16", bufs=8) as pg16,
                tc.tile_pool(name="pout", bufs=4) as pout,
                tc.tile_pool(name="praw", bufs=2) as praw,
            ):
                # prelude: one-hots + gather indices for all batches
                ohTs, its = [], []
                for b in range(BPC):
                    pos_b = pposb.tile([128, S], F32, tag="posb")
                    nc.gpsimd.dma_start(
                        pos_b[:],
                        pos_d.ap()[b].unsqueeze(0).partition_broadcast(128),
                    )
                    ohT = pohT.tile([128, KB, S], BF16, tag="ohT")
                    for c in range(KB):
                        nc.vector.tensor_scalar(
                            ohT[:, c, :], pos_b[:], iota2[:, c : c + 1], None, op0=EQ
                        )
                    ohTs.append(ohT)
                    for ci in range(S // GCH):
                        it = pg16.tile([128, GCH // 16], I16, tag="g16")
                        nc.gpsimd.dma_start(it[:], pos16_d.ap()[b, ci])
                        its.append(it)

                # sigmoid + ALPHA scale in quarter slices, scalar-engine queue
                QS = KB // 4  # 2 v1-blocks per quarter
                for qi in range(4):
                    h, qh = qi // 2, qi % 2
                    raw = praw.tile([128, QS, N_POS], F32, tag="raw")
                    nc.scalar.dma_start(
                        raw[:],
                        hist_out[h]
                        .ap()[qh * 256 : (qh + 1) * 256, :]
                        .rearrange("(kb q) v -> q kb v", q=128),
                    )
                    sg = praw.tile([128, QS, N_POS], F32, tag="sg")
                    nc.scalar.activation(
                        sg[:], raw[:], mybir.ActivationFunctionType.Sigmoid
                    )
                    sc = praw.tile([128, QS, N_POS], BF16, tag="sc")
                    nc.vector.tensor_scalar_mul(sc[:], sg[:], ALPHA)
                    nc.scalar.dma_start(
                        score_bf.ap()[qi * 256 : (qi + 1) * 256, :].rearrange(
                            "(kb q) v -> q kb v", q=128
                        ),
                        sc[:],
                    )

                # all gathers up front (gpsimd queue), then the matmul stream
                wt_cs = []
                for b in range(BPC):
                    for ci in range(S // GCH):
                        wt = pwt.tile([128, KB, GCH], BF16, tag="wt")
                        nc.gpsimd.dma_gather(
                            out_ap=wt[:],
                            in_ap=score_bf.ap(),
                            idxs_ap=its[b * (S // GCH) + ci][:],
                            num_idxs=GCH,
                            num_idxs_reg=GCH,
                            elem_size=N_POS,
                            transpose=True,
                        )
                        wt_cs.append(wt)

                for b in range(BPC):
                    o_view = o_d.ap()[b].rearrange("(mb q) n -> q mb n", q=128)
                    s_view = s_d.ap()[b].rearrange("(mb q) n -> q mb n", q=128)
                    for half in range(2):
                        mlo = half * (KB // 2)
                        s_t = pst.tile([128, KB // 2, S], F32, tag="st")
                        nc.scalar.dma_start(
                            s_t[:], s_view[:, mlo : mlo + KB // 2, :]
                        )
                        # MMb: T[p1, p2] = sum_v2 Wt[v2,p1] * onehotPT[v2,p2]
                        for m in range(mlo, mlo + KB // 2):
                            ci, off = m // (GCH // 128), (m % (GCH // 128)) * 128
                            wt = wt_cs[b * (S // GCH) + ci]
                            for n in range(2):
                                pt = pps.tile([128, 512], F32, tag="ps")
                                for k in range(KB):
                                    nc.tensor.matmul(
                                        pt[:],
                                        wt[:, k, off : off + 128],
                                        ohTs[b][:, k, n * 512 : (n + 1) * 512],
                                        start=(k == 0),
                                        stop=(k == KB - 1),
                                    )
                                ot = pout.tile([128, 512], F32, tag="ot")
                                nc.vector.tensor_tensor(
                                    ot[:],
                                    s_t[:, m - mlo, n * 512 : (n + 1) * 512],
                                    pt[:],
                                    op=ADD,
                                )
                                oeng = nc.sync if n == 0 else nc.scalar
                                oeng.dma_start(
                                    o_view[:, m, n * 512 : (n + 1) * 512], ot[:]
                                )
    nc.finalize()
    return nc


def _get_nc():
    if "nc" not in _CACHE:
        _CACHE["nc"] = _build_nc()
    return _CACHE["nc"]


def _pos16(pos_shard):
    # wrapped-in-16-partitions layout per GCH-chunk, replicated across the
    # 8 gpsimd 16-partition groups
    out = np.empty((BPC, S // GCH, 128, GCH // 16), np.int16)
    for b in range(BPC):
        for ci in range(S // GCH):
            vals = pos_shard[b, ci * GCH : (ci + 1) * GCH].reshape(GCH // 16, 16)
            out[b, ci] = np.tile(vals.T, (8, 1))
    return out


def _make_in_maps(s_arc, a_arc, adds, pos):
    pos_i = np.asarray(pos).astype(np.int64)
    in_maps = []
    for c in range(NCORES):
        sl = slice(BPC * c, BPC * (c + 1))
        in_maps.append(
            {
                "a": np.ascontiguousarray(a_arc[sl]).astype(_bf16),
                "s": np.ascontiguousarray(s_arc[sl], dtype=np.float32),
                "adds": np.ascontiguousarray(adds[sl], dtype=np.float32),
                "pos": np.ascontiguousarray(pos[sl], dtype=np.float32),
                "pos16": _pos16(pos_i[sl]),
            }
        )
    return in_maps


def _run(in_maps, trace=False, **kwargs):
    return run_bass_kernel_spmd(
        _get_nc(), in_maps, core_ids=list(range(NCORES)), trace=trace, **kwargs
    )


def kernel(s_arc, a_arc, adds, pos):
    s_arc = np.asarray(s_arc)
    a_arc = np.asarray(a_arc)
    assert s_arc.shape == (B, S, S) and a_arc.shape == (B, S, S), (
        s_arc.shape,
        a_arc.shape,
    )
    in_maps = _make_in_maps(s_arc, a_arc, adds, pos)
    try:
        res = _run(in_maps, trace=False)
    except Exception:
        res = _run(in_maps, trace=False)
    out = np.empty((B, S, S), dtype=np.float32)
    for c in range(NCORES):
        out[BPC * c : BPC * (c + 1)] = res.results[c]["o"]
    return out
